# revision 40
# baseline (speedup 1.0000x reference)
"""HGT (2-type, 2-relation, 2-layer) Bass kernel for 8 Trainium2 cores.

dst-major degree-bucketed layout: nodes of each type are assigned to cores
(8192 lowest-out-degree nodes, incl. pads, land in table positions >= 32768
so the int16-indexed "hi" gather region sees few edges), and sorted within
each core's 5120-row shard by (lo in-degree, hi in-degree) desc. Each dst
node owns one partition row; its in-edges occupy columns of a [128, E]
slot rectangle per 128-dst group (E = per-group max degree, lo/hi regions
separate). K||V source rows are gathered per slot in bf16 from an
AllGathered per-type table (4 SWDGE queues, chunk prefetch); attention
softmax + weighted sum are pure per-partition vector ops. Each
(relation, layer) phase runs in two passes (softmax -> at buffer, then
gelu+alin) so the Activation engine never swaps tables per group. Biases
are Act-initialized into PSUM; the skip connection is a matmul against
oms*I accumulated into the same PSUM bank.
"""
import math
import os
import sys

import numpy as np

sys.path.insert(0, "/opt/trn_rl_repo")

H, D, C, L = 4, 32, 128, 2
INV_SQRT_D = 1.0 / math.sqrt(D)
P = 128
NCORES = 8
SHARD = 5120
NGRP = SHARD // P          # 40
NPAD = NCORES * SHARD      # 40960
LO_LIM = 6 * SHARD         # 30720: cores 0-5 are the "lo" gather region,
                           # cores 6-7 "hi"; no core straddles the boundary
N_REAL = 40000
CAP_LO = 24                # gather chunk capacity in slot-columns
CAP_HI = 10
NQ = int(os.environ.get("NQ", "4"))          # SWDGE queues for gathers
SAFE_BIAS = bool(os.environ.get("SAFE_BIAS"))  # ones-matmul bias fallback

LAST_RESULT = None


def _fold_weights(ins):
    """Fold a_rel/m_rel into k/v, p_rel/sqrt(D) into q, sigmoid(skip) into
    a_lin. Produces combined kvq [C, 384] weights; biases tiled to [P, w]."""
    f = {}
    for l in range(L):
        for t in range(2):
            kw = np.asarray(ins["k_w"][l, t])
            kb = np.asarray(ins["k_b"][l, t])
            vw = np.asarray(ins["v_w"][l, t])
            vb = np.asarray(ins["v_b"][l, t])
            ar = np.asarray(ins["a_rel"][l, t])   # rel t has src type t
            mr = np.asarray(ins["m_rel"][l, t])
            wk = np.zeros((C, C), np.float32)
            wv = np.zeros((C, C), np.float32)
            bk = np.zeros(C, np.float32)
            bv = np.zeros(C, np.float32)
            for h in range(H):
                sl = slice(h * D, (h + 1) * D)
                wk[:, sl] = kw[:, sl] @ ar[h]
                wv[:, sl] = vw[:, sl] @ mr[h]
                bk[sl] = kb[sl] @ ar[h]
                bv[sl] = vb[sl] @ mr[h]
            r_dst = 1 - t                          # type t is dst of rel 1-t
            pr = np.asarray(ins["p_rel"][l, r_dst]) * INV_SQRT_D
            scale = np.repeat(pr, D)
            wq = (np.asarray(ins["q_w"][l, t]) * scale[None, :]).astype(np.float32)
            bq = (np.asarray(ins["q_b"][l, t]) * scale).astype(np.float32)
            f[f"Wkvq{l}{t}"] = np.concatenate([wk, wv, wq], axis=1)       # [C, 384]
            f[f"Bkvq{l}{t}"] = np.tile(np.concatenate([bk, bv, bq])[None, :], (P, 1))
            s = 1.0 / (1.0 + math.exp(-float(np.asarray(ins["skip"][l, t]))))
            f[f"Wal{l}{t}"] = (np.asarray(ins["a_lin_w"][l, t]) * s).astype(np.float32)
            f[f"Bal{l}{t}"] = np.tile((np.asarray(ins["a_lin_b"][l, t]) * s)[None, :],
                                      (P, 1)).astype(np.float32)
            f[f"OmsI{l}{t}"] = ((1.0 - s) * np.eye(P)).astype(np.float32)
    f["Wina"] = np.asarray(ins["lin_a_w"]).astype(np.float32)
    f["Bina"] = np.tile(np.asarray(ins["lin_a_b"])[None, :], (P, 1)).astype(np.float32)
    f["Winb"] = np.asarray(ins["lin_b_w"]).astype(np.float32)
    f["Binb"] = np.tile(np.asarray(ins["lin_b_b"])[None, :], (P, 1)).astype(np.float32)
    return f


def _assign_positions(out_deg):
    """Assign nodes (incl. pads) to table positions. Lowest out-degree 8192
    nodes go to positions >= LO_LIM. Returns node_at_pos [NPAD] int64."""
    order = np.argsort(out_deg, kind="stable")
    hi_nodes = order[: NPAD - LO_LIM]
    lo_nodes = order[NPAD - LO_LIM:]
    node_at_pos = np.empty(NPAD, np.int64)
    node_at_pos[:LO_LIM] = np.sort(lo_nodes)
    node_at_pos[LO_LIM:] = np.sort(hi_nodes)
    return node_at_pos


def _sort_within_cores(node_at_pos, lo_indeg, hi_indeg):
    """Reorder positions within each core's shard by (lo,hi) in-degree desc,
    keeping the lo/hi membership boundary inside the shard fixed."""
    out = node_at_pos.copy()
    for c in range(NCORES):
        s0, s1 = c * SHARD, (c + 1) * SHARD
        cut = min(max(LO_LIM - s0, 0), SHARD)
        for a, b in ((s0, s0 + cut), (s0 + cut, s1)):
            if b - a <= 1:
                continue
            seg = out[a:b]
            key = np.lexsort((-hi_indeg[seg], -lo_indeg[seg]))
            out[a:b] = seg[key]
    return out


def _prep_edges(edge, pos_src, pos_dst):
    """Build the dst-major slot schedule for one relation.

    Returns (Elo[NGRP], Ehi[NGRP], idx_w[8], mask[8], TB)."""
    src = pos_src[np.asarray(edge[0]).astype(np.int64)]
    dst = pos_dst[np.asarray(edge[1]).astype(np.int64)]
    core = dst // SHARD
    row = dst % SHARD
    g = row // P
    part = row % P
    lo = src < LO_LIM

    Elo = np.zeros(NGRP, np.int64)
    Ehi = np.zeros(NGRP, np.int64)
    per_core = []
    for c in range(NCORES):
        m = core == c
        key = (g[m] * P + part[m]) * 2 + (~lo[m])
        cnt = np.bincount(key, minlength=SHARD * 2).reshape(NGRP, P, 2)
        Elo = np.maximum(Elo, cnt[:, :, 0].max(1))
        Ehi = np.maximum(Ehi, cnt[:, :, 1].max(1))
        per_core.append((src[m], g[m], part[m], lo[m]))
    col_lo = np.concatenate([[0], np.cumsum(Elo)])
    col_hi = np.concatenate([[0], np.cumsum(Ehi)]) + col_lo[-1]
    TB = int(col_lo[-1] + Ehi.sum())

    idx_ws, masks = [], []
    for c in range(NCORES):
        s, gg, pp, ll = per_core[c]
        idx = np.zeros((P, TB), np.int16)
        mask = np.zeros((P, TB), np.float32)
        for want_lo, colbase, idx_off in ((True, col_lo, 0), (False, col_hi, LO_LIM)):
            mm = ll if want_lo else ~ll
            ss, gs, ps = s[mm], gg[mm], pp[mm]
            order = np.lexsort((ps, gs))
            ss, gs, ps = ss[order], gs[order], ps[order]
            key = gs * P + ps
            first = np.concatenate([[True], key[1:] != key[:-1]])
            runstart = np.maximum.accumulate(np.where(first, np.arange(len(key)), 0))
            rank = np.arange(len(key)) - runstart
            cols = colbase[gs] + rank
            idx[ps, cols] = (ss - idx_off).astype(np.int16)
            mask[ps, cols] = 1.0
        SL = TB * P
        flat = idx.T.reshape(SL)
        idx_ws.append(np.tile(flat.reshape(SL // 16, 16).T, (8, 1)).copy())
        masks.append(mask.copy())
    return Elo.tolist(), Ehi.tolist(), idx_ws, masks, TB


def _pack_chunks(E_list, colbase, cap):
    """Pack consecutive groups into gather chunks of at most cap columns."""
    chunks, loc = [], {}
    cur0, cols = None, 0
    for g, E in enumerate(E_list):
        if E == 0:
            continue
        if cur0 is not None and cols + E > cap:
            chunks.append((cur0, cols))
            cur0, cols = None, 0
        if cur0 is None:
            cur0 = int(colbase[g])
        loc[g] = (len(chunks), cols)
        cols += int(E)
    if cur0 is not None:
        chunks.append((cur0, cols))
    return chunks, loc


def kernel(**ins):
    global LAST_RESULT
    import concourse.bass as bass
    import concourse.tile as tile
    from concourse import bacc, mybir
    from concourse.bass_utils import run_bass_kernel_spmd
    from concourse.masks import make_identity

    FP = mybir.dt.float32
    BF = mybir.dt.bfloat16
    I16 = mybir.dt.int16
    AL = mybir.AluOpType
    AF = mybir.ActivationFunctionType

    f = _fold_weights(ins)

    edge_ab = np.asarray(ins["edge_ab"])
    edge_ba = np.asarray(ins["edge_ba"])
    out_deg_a = np.bincount(edge_ab[0], minlength=NPAD)
    out_deg_b = np.bincount(edge_ba[0], minlength=NPAD)
    nap_a = _assign_positions(out_deg_a)
    nap_b = _assign_positions(out_deg_b)
    pos_a = np.empty(NPAD, np.int64)
    pos_a[nap_a] = np.arange(NPAD)
    pos_b = np.empty(NPAD, np.int64)
    pos_b[nap_b] = np.arange(NPAD)
    lo_in_a = np.bincount(edge_ba[1], weights=(pos_b[edge_ba[0]] < LO_LIM), minlength=NPAD)
    hi_in_a = np.bincount(edge_ba[1], weights=(pos_b[edge_ba[0]] >= LO_LIM), minlength=NPAD)
    lo_in_b = np.bincount(edge_ab[1], weights=(pos_a[edge_ab[0]] < LO_LIM), minlength=NPAD)
    hi_in_b = np.bincount(edge_ab[1], weights=(pos_a[edge_ab[0]] >= LO_LIM), minlength=NPAD)
    nap_a = _sort_within_cores(nap_a, lo_in_a, hi_in_a)
    nap_b = _sort_within_cores(nap_b, lo_in_b, hi_in_b)
    pos_a[nap_a] = np.arange(NPAD)
    pos_b[nap_b] = np.arange(NPAD)

    Elo0, Ehi0, idx0, mask0, TB0 = _prep_edges(edge_ab, pos_a, pos_b)
    Elo1, Ehi1, idx1, mask1, TB1 = _prep_edges(edge_ba, pos_b, pos_a)
    rel_meta = [(Elo0, Ehi0, TB0), (Elo1, Ehi1, TB1)]
    EC = int(max(max(Elo0), max(Ehi0), max(Elo1), max(Ehi1)))

    xa = np.zeros((NPAD, 64), np.float32)
    xa[:N_REAL] = np.asarray(ins["x_a"]).astype(np.float32)
    xb = np.zeros((NPAD, 32), np.float32)
    xb[:N_REAL] = np.asarray(ins["x_b"]).astype(np.float32)
    xaT = np.ascontiguousarray(xa[nap_a].T)
    xbT = np.ascontiguousarray(xb[nap_b].T)

    nc = bacc.Bacc("TRN2", target_bir_lowering=False, debug=False,
                   num_devices=NCORES, num_swdge_queues=NQ)

    # ---- DRAM tensors ----
    t_xasT = nc.dram_tensor("xasT", [64, SHARD], FP, kind="ExternalInput").ap()
    t_xbsT = nc.dram_tensor("xbsT", [32, SHARD], FP, kind="ExternalInput").ap()
    wnames = ["Wina", "Winb", "Bina", "Binb"]
    for l in range(L):
        for t in range(2):
            wnames += [f"Wkvq{l}{t}", f"Bkvq{l}{t}", f"Wal{l}{t}", f"Bal{l}{t}",
                       f"OmsI{l}{t}"]
    t_w = {n: nc.dram_tensor(n, list(f[n].shape), FP, kind="ExternalInput").ap()
           for n in wnames}
    t_idx = [nc.dram_tensor(f"idx{r}", [P, rel_meta[r][2] * 8], I16,
                            kind="ExternalInput").ap() for r in range(2)]
    t_mask = [nc.dram_tensor(f"mask{r}", [P, rel_meta[r][2]], BF,
                             kind="ExternalInput").ap() for r in range(2)]
    t_kvsh = [[nc.dram_tensor(f"kvsh{t}{l}", [SHARD, 2 * C], BF) for l in range(L)]
              for t in range(2)]
    t_ag = [[nc.dram_tensor(f"ag{t}{l}", [NCORES, SHARD, 2 * C], BF,
                            addr_space="Shared") for l in range(L)] for t in range(2)]
    t_out = [nc.dram_tensor(f"out{t}", [SHARD, C], FP, kind="ExternalOutput").ap()
             for t in range(2)]

    with tile.TileContext(nc) as tc:
        cpool_cm = tc.tile_pool(name="const", bufs=1)
        cpool = cpool_cm.__enter__()
        ident = cpool.tile([P, P], FP)
        make_identity(nc, ident[:])
        w_sb = {}
        for n in wnames:
            w_sb[n] = cpool.tile(list(f[n].shape), FP, name=n, tag=n)
            nc.sync.dma_start(out=w_sb[n][:], in_=t_w[n][:])
        idx_sb = []
        mask_sb = []
        for r in range(2):
            TB = rel_meta[r][2]
            it = cpool.tile([P, TB * 8], I16, name=f"idxsb{r}", tag=f"idxsb{r}")
            nc.sync.dma_start(out=it[:], in_=t_idx[r][:])
            idx_sb.append(it)
            mtf = cpool.tile([P, TB], BF, name=f"masksb{r}", tag=f"masksb{r}")
            nc.sync.dma_start(out=mtf[:], in_=t_mask[r][:])
            mask_sb.append(mtf)
        q_sb = [cpool.tile([P, NGRP, C], BF, name=f"qsb{t}", tag=f"qsb{t}")
                for t in range(2)]
        q_sb2 = [cpool.tile([P, NGRP, C], BF, name=f"qsb2{t}", tag=f"qsb2{t}")
                 for t in range(2)]
        xT_sb = [cpool.tile([P, NGRP, P], FP, name=f"xTsb{t}", tag=f"xTsb{t}")
                 for t in range(2)]

        ones_sb = cpool.tile([1, P], FP, name="ones", tag="ones")
        nc.vector.memset(ones_sb[:], 1.0)
        eps_sb = cpool.tile([P, H], FP, name="eps", tag="eps")
        nc.vector.memset(eps_sb[:], 1e-16)

        def psum_bias(ps_ap, bias_name, width):
            if SAFE_BIAS:
                nc.tensor.matmul(out=ps_ap, lhsT=ones_sb[:],
                                 rhs=w_sb[bias_name][0:1, 0:width],
                                 start=True, stop=False)
            else:
                nc.scalar.activation(out=ps_ap, in_=w_sb[bias_name][:, 0:width],
                                     func=AF.Copy)

        # ---------- layer-0 shard projection: x0, q0, kv0 ----------
        def proj0(t, xT_ap, Win, Bin, src_pool, ps_pool, out_pool):
            DIN = xT_ap.shape[0]
            for j in range(NGRP):
                lhs = src_pool.tile([DIN, P], FP, tag="lhs0")
                nc.sync.dma_start(out=lhs[:], in_=xT_ap[:, j * P:(j + 1) * P])
                ps1 = ps_pool.tile([P, C], FP, space="PSUM", tag="ps1")
                psum_bias(ps1[:], Bin, C)
                nc.tensor.matmul(out=ps1[:], lhsT=lhs[:], rhs=w_sb[Win][:],
                                 start=False, stop=True)
                x0 = out_pool.tile([P, C], FP, tag="x0")
                nc.scalar.activation(out=x0[:], in_=ps1[:], func=AF.Relu)
                pst = ps_pool.tile([P, P], FP, space="PSUM", tag="pst0")
                nc.tensor.transpose(out=pst[:], in_=x0[:], identity=ident[:])
                nc.scalar.activation(out=xT_sb[t][:, j, :], in_=pst[:], func=AF.Copy)
                ps2 = ps_pool.tile([P, 3 * C], FP, space="PSUM", tag="ps2")
                psum_bias(ps2[:], f"Bkvq0{t}", 3 * C)
                nc.tensor.matmul(out=ps2[:], lhsT=xT_sb[t][:, j, :],
                                 rhs=w_sb[f"Wkvq0{t}"][:], start=False, stop=True)
                kvt = out_pool.tile([P, 2 * C], BF, tag="kvt")
                nc.scalar.activation(out=kvt[:], in_=ps2[:, 0:2 * C], func=AF.Copy)
                nc.scalar.activation(out=q_sb[t][:, j, :], in_=ps2[:, 2 * C:3 * C],
                                     func=AF.Copy)
                nc.sync.dma_start(out=t_kvsh[t][0].ap()[j * P:(j + 1) * P, :],
                                  in_=kvt[:])

        def allgather(t, l):
            nc.gpsimd.collective_compute(
                "AllGather", mybir.AluOpType.bypass,
                replica_groups=[list(range(NCORES))],
                ins=[t_kvsh[t][l].ap()[:]], outs=[t_ag[t][l].ap()[:]],
            )

        with (
            tc.tile_pool(name="p0src", bufs=6) as src_pool,
            tc.tile_pool(name="p0ps", bufs=2, space="PSUM") as ps_pool,
            tc.tile_pool(name="p0out", bufs=6) as out_pool,
        ):
            proj0(0, t_xasT, "Wina", "Bina", src_pool, ps_pool, out_pool)
            allgather(0, 0)
            proj0(1, t_xbsT, "Winb", "Binb", src_pool, ps_pool, out_pool)
            allgather(1, 0)

        # ---------- fused attention + alin per (relation, layer) ----------
        def att_alin(r, l, pre=None):
            # all-engine barrier: no gather may start before the AllGather
            # that produced this relation's table has fully completed
            tc.strict_bb_all_engine_barrier()
            if pre is not None:
                pre()   # issue the NEXT layer's AllGather after the barrier
                        # so this phase's barrier does not wait for it
            d = 1 - r
            Elo, Ehi, TB = rel_meta[r]
            col_lo = np.concatenate([[0], np.cumsum(Elo)])
            col_hi = np.concatenate([[0], np.cumsum(Ehi)]) + col_lo[-1]
            lo_chunks, lo_loc = _pack_chunks(Elo, col_lo, CAP_LO)
            hi_chunks, hi_loc = _pack_chunks(Ehi, col_hi, CAP_HI)
            table = t_ag[r][l].ap().rearrange("c s k -> (c s) k")
            idxt = idx_sb[r]
            maskt = mask_sb[r]
            qt = q_sb[d] if l == 0 else q_sb2[d]
            qno = [0]
            with (
                tc.tile_pool(name=f"glo{r}{l}", bufs=2) as glo_pool,
                tc.tile_pool(name=f"ghi{r}{l}", bufs=2) as ghi_pool,
                tc.tile_pool(name=f"wk{r}{l}", bufs=3) as wk_pool,
                tc.tile_pool(name=f"sm{r}{l}", bufs=4) as sm_pool,
                tc.tile_pool(name=f"at{r}{l}", bufs=8) as at_pool,
                tc.tile_pool(name=f"aps{r}{l}", bufs=2, space="PSUM") as aps,
            ):
                tiles = {}

                def get_chunk(region, ci):
                    key = (region, ci)
                    if key in tiles:
                        return tiles[key]
                    chunks, pool, cap = (
                        (lo_chunks, glo_pool, CAP_LO) if region == 0
                        else (hi_chunks, ghi_pool, CAP_HI))
                    if ci >= len(chunks):
                        return None
                    col0, ncols = chunks[ci]
                    gt = pool.tile([P, cap, 2 * C], BF, tag=f"ch{region}")
                    in_ap = (table[0:LO_LIM, :] if region == 0
                             else table[LO_LIM:NPAD, :])
                    nc.gpsimd.dma_gather(
                        out_ap=gt[:, 0:ncols, :], in_ap=in_ap,
                        idxs_ap=idxt[:, col0 * 8:(col0 + ncols) * 8],
                        num_idxs=ncols * P, num_idxs_reg=ncols * P,
                        elem_size=2 * C, single_packet=False,
                        queue_num=qno[0] % NQ,
                    )
                    qno[0] += 1
                    tiles[key] = gt
                    return gt

                # ---- pass A (per group): softmax + weighted sums -> at tile ----
                at_tiles = {}

                def pass_a(g):
                    regions = []
                    if Elo[g] > 0:
                        ci, off = lo_loc[g]
                        regions.append((0, ci, off, int(Elo[g]), int(col_lo[g])))
                    if Ehi[g] > 0:
                        ci, off = hi_loc[g]
                        regions.append((1, ci, off, int(Ehi[g]), int(col_hi[g])))
                    s_parts, wvs_parts = [], []
                    for region, ci, off, E, gcol in regions:
                        gt = get_chunk(region, ci)
                        get_chunk(region, ci + 1)   # prefetch
                        kt = gt[:, off:off + E, 0:C]
                        vt = gt[:, off:off + E, C:2 * C]
                        lp = wk_pool.tile([P, EC, C], BF, tag="lp")
                        nc.vector.tensor_tensor(
                            out=lp[:, 0:E, :], in0=kt,
                            in1=qt[:, g, :].rearrange("p (o c) -> p o c", o=1)
                                .to_broadcast([P, E, C]),
                            op=AL.mult)
                        z = sm_pool.tile([P, EC * H], FP, tag="z")
                        nc.vector.tensor_reduce(
                            out=z[:, 0:E * H],
                            in_=lp[:, 0:E, :].rearrange("p e (h dd) -> p (e h) dd", h=H),
                            axis=mybir.AxisListType.X, op=AL.add)
                        ze = sm_pool.tile([P, EC * H], BF, tag="ze")
                        nc.scalar.activation(out=ze[:, 0:E * H], in_=z[:, 0:E * H],
                                             func=AF.Exp)
                        zem = sm_pool.tile([P, EC, H], BF, tag="zem")
                        nc.vector.tensor_tensor(
                            out=zem[:, 0:E, :],
                            in0=ze[:, 0:E * H].rearrange("p (e h) -> p e h", h=H),
                            in1=maskt[:, gcol:gcol + E]
                                .rearrange("p (e o) -> p e o", o=1)
                                .to_broadcast([P, E, H]),
                            op=AL.mult)
                        sp = sm_pool.tile([P, H], FP, tag=f"sp{len(s_parts)}")
                        nc.vector.tensor_reduce(
                            out=sp[:], in_=zem[:, 0:E, :].rearrange("p e h -> p h e"),
                            axis=mybir.AxisListType.X, op=AL.add)
                        s_parts.append(sp)
                        wv = wk_pool.tile([P, EC, C], FP, tag="wv")
                        nc.vector.tensor_tensor(
                            out=wv[:, 0:E, :].rearrange("p e (h dd) -> p e h dd", h=H),
                            in0=vt.rearrange("p e (h dd) -> p e h dd", h=H),
                            in1=zem[:, 0:E, :].rearrange("p e (h o) -> p e h o", o=1)
                                .to_broadcast([P, E, H, D]),
                            op=AL.mult)
                        # contiguous tree-halving reduction over e
                        n = E
                        while n > 1:
                            hh = n // 2
                            nc.vector.tensor_tensor(
                                out=wv[:, 0:hh, :], in0=wv[:, 0:hh, :],
                                in1=wv[:, n - hh:n, :], op=AL.add)
                            n -= hh
                        wvs_parts.append(wv)
                    if len(regions) == 0:
                        s = sm_pool.tile([P, H], FP, tag="s")
                        nc.vector.memset(s[:], 0.0)
                        wvs = sm_pool.tile([P, C], FP, tag="wvs")
                        nc.vector.memset(wvs[:], 0.0)
                        wvs = wvs[:]
                    elif len(regions) == 1:
                        s = s_parts[0]
                        wvs = wvs_parts[0][:, 0, :]
                    else:
                        s = sm_pool.tile([P, H], FP, tag="s")
                        nc.vector.tensor_tensor(out=s[:], in0=s_parts[0][:],
                                                in1=s_parts[1][:], op=AL.add)
                        wvs = sm_pool.tile([P, C], FP, tag="wvs")
                        nc.vector.tensor_tensor(out=wvs[:], in0=wvs_parts[0][:, 0, :],
                                                in1=wvs_parts[1][:, 0, :], op=AL.add)
                        wvs = wvs[:]
                    den = sm_pool.tile([P, H], FP, tag="den")
                    nc.vector.tensor_tensor(out=den[:], in0=s[:], in1=eps_sb[:],
                                            op=AL.add)
                    rec = sm_pool.tile([P, H], FP, tag="rec")
                    nc.vector.reciprocal(rec[:], den[:])
                    att = at_pool.tile([P, C], BF, tag="at")
                    nc.vector.tensor_tensor(
                        out=att[:].rearrange("p (h dd) -> p h dd", h=H),
                        in0=wvs.rearrange("p (h dd) -> p h dd", h=H),
                        in1=rec[:].rearrange("p (h o) -> p h o", o=1)
                            .to_broadcast([P, H, D]),
                        op=AL.mult)
                    at_tiles[g] = att

                # ---- pass B (per group): gelu + a_lin + skip (+ kvq) ----
                def pass_b(g):
                    gl = sm_pool.tile([P, C], FP, tag="gl")
                    nc.scalar.activation(out=gl[:], in_=at_tiles[g][:], func=AF.Gelu)
                    pst = aps.tile([P, P], FP, space="PSUM", tag="pst")
                    nc.tensor.transpose(out=pst[:], in_=gl[:], identity=ident[:])
                    glT = sm_pool.tile([P, P], FP, tag="glT")
                    nc.vector.tensor_copy(glT[:], pst[:])
                    psn = aps.tile([P, C], FP, space="PSUM", tag="psn")
                    psum_bias(psn[:], f"Bal{l}{d}", C)
                    nc.tensor.matmul(out=psn[:], lhsT=glT[:], rhs=w_sb[f"Wal{l}{d}"][:],
                                     start=False, stop=False)
                    nc.tensor.matmul(out=psn[:], lhsT=xT_sb[d][:, g, :],
                                     rhs=w_sb[f"OmsI{l}{d}"][:], start=False, stop=True)
                    nw = sm_pool.tile([P, C], FP, tag="nw")
                    nc.scalar.activation(out=nw[:], in_=psn[:], func=AF.Copy)
                    if l == 0:
                        pst2 = aps.tile([P, P], FP, space="PSUM", tag="pst2")
                        nc.tensor.transpose(out=pst2[:], in_=nw[:], identity=ident[:])
                        nc.scalar.activation(out=xT_sb[d][:, g, :], in_=pst2[:],
                                             func=AF.Copy)
                        ps2 = aps.tile([P, 3 * C], FP, space="PSUM", tag="ps2")
                        psum_bias(ps2[:], f"Bkvq1{d}", 3 * C)
                        nc.tensor.matmul(out=ps2[:], lhsT=xT_sb[d][:, g, :],
                                         rhs=w_sb[f"Wkvq1{d}"][:], start=False,
                                         stop=True)
                        kvt = sm_pool.tile([P, 2 * C], BF, tag="kvt")
                        nc.scalar.activation(out=kvt[:], in_=ps2[:, 0:2 * C],
                                             func=AF.Copy)
                        nc.scalar.activation(out=q_sb2[d][:, g, :],
                                             in_=ps2[:, 2 * C:3 * C], func=AF.Copy)
                        nc.sync.dma_start(out=t_kvsh[d][1].ap()[g * P:(g + 1) * P, :],
                                          in_=kvt[:])
                    else:
                        nc.sync.dma_start(out=t_out[d][g * P:(g + 1) * P, :],
                                          in_=nw[:])

                # batch-interleaved driver: pass B trails pass A by one batch
                BATCH = 6
                done_b = 0
                for b0 in range(0, NGRP, BATCH):
                    for g in range(b0, min(b0 + BATCH, NGRP)):
                        pass_a(g)
                    if b0 > 0:
                        for g in range(done_b, b0):
                            pass_b(g)
                        done_b = b0
                for g in range(done_b, NGRP):
                    pass_b(g)

        att_alin(0, 0)
        att_alin(1, 0, pre=lambda: allgather(1, 1))
        att_alin(1, 1, pre=lambda: allgather(0, 1))
        att_alin(0, 1)
        cpool_cm.__exit__(None, None, None)

    nc.compile()

    import ml_dtypes
    in_maps = []
    for c in range(NCORES):
        m = {"xasT": np.ascontiguousarray(xaT[:, c * SHARD:(c + 1) * SHARD]),
             "xbsT": np.ascontiguousarray(xbT[:, c * SHARD:(c + 1) * SHARD]),
             "idx0": idx0[c], "mask0": mask0[c].astype(ml_dtypes.bfloat16),
             "idx1": idx1[c], "mask1": mask1[c].astype(ml_dtypes.bfloat16)}
        for n in wnames:
            m[n] = np.ascontiguousarray(f[n])
        in_maps.append(m)

    res = run_bass_kernel_spmd(
        nc, in_maps, core_ids=list(range(NCORES)),
        trace=bool(os.environ.get("BASS_TRACE")),
    )
    LAST_RESULT = res
    outa_p = np.concatenate([res.results[c]["out0"] for c in range(NCORES)])
    outb_p = np.concatenate([res.results[c]["out1"] for c in range(NCORES)])
    outa = np.empty((N_REAL, C), np.float32)
    outb = np.empty((N_REAL, C), np.float32)
    real_a = nap_a < N_REAL
    real_b = nap_b < N_REAL
    outa[nap_a[real_a]] = outa_p[real_a]
    outb[nap_b[real_b]] = outb_p[real_b]
    return outa, outb
